# revision 6
# baseline (speedup 1.0000x reference)
"""DETR3D decoder layer on 8 Trainium2 NeuronCores (Bass/Tile).

Sharding: 2 batches x 4 query-shards of 256 (queries padded 900->1024).
Each core owns 256 queries end-to-end (self-attn, cross-attn fusion, FFN)
-- no collectives.  Cross-attention feature sampling is done with
DRAM-source dma_gather: the host pre-transposes each camera's feature
pyramid to channel-last bf16 ([sum HW, 256] rows), precomputes int16
gather row indices (bilinear x-corner pairs: one index fetches both
x0/x1 points = 1KB) and the static bilinear/mask weights; the device
multiplies them by the data-dependent attention weights (sigmoid) and
accumulates with per-partition-scalar vector ops (queries on partitions).
"""

import math
import numpy as np

B, Q, C, NH, DH, NCAM, NL, FFD = 2, 900, 256, 8, 32, 6, 3, 1024
QP = 1024          # padded queries per batch
QL = 256           # queries per core
LVLS = [(116, 200), (58, 100), (29, 50)]
HWL = [h * w for h, w in LVLS]             # 23200, 5800, 1450
CAMROWS = sum(HWL) + 14                    # 30450 + pad -> 30464
LVL_BASE = [0, HWL[0], HWL[0] + HWL[1]]
IMG_H, IMG_W = 928.0, 1600.0
PCS = [102.4, 102.4, 8.0]
PCO = [-51.2, -51.2, -5.0]
EPS = 1e-5
NCORES = 8
NIDX = NL * 2 * QL                         # 1536 per camera gather
NJ = NL * 2 * 2                            # 12 blocks of 128 idx per cam

_BUILT = {}


def _build():
    import concourse.bass as bass
    import concourse.bacc as bacc
    import concourse.tile as tile
    from concourse import mybir

    nc = bacc.Bacc("TRN2", target_bir_lowering=False, debug=False,
                   num_devices=NCORES)
    with tile.TileContext(nc) as tc:
        _emit(tc, nc, bass, mybir)
    nc.compile()
    return nc


def _emit(tc, nc, bass, mybir):
    import contextlib
    F32 = mybir.dt.float32
    BF = mybir.dt.bfloat16
    I16 = mybir.dt.int16
    AF = mybir.ActivationFunctionType
    AL = mybir.AluOpType
    ts_ = bass.ts
    AWC = NCAM * NL   # 18

    def din(name, shape, dt=F32):
        return nc.dram_tensor(name, list(shape), dt, kind="ExternalInput").ap()

    # ------------- external inputs -------------
    xposbf_d = din("xposbf", (128, 2, QP), BF)
    xvbf_d = din("xvbf", (128, 2, QP), BF)
    xlocbf_d = din("xlocbf", (128, 2, QL), BF)
    qrows_d = din("qrows_loc", (QL, C))
    refrows_d = din("refrows_loc", (QL, 3))
    camf_d = [din(f"camf{n}", (CAMROWS, C), BF) for n in range(NCAM)]
    camidx_d = din("camidx", (32, NCAM, NIDX // 16), I16)
    hostw_d = din("hostw", (128, 2, NCAM * NL * 4))
    wqkvT_d = din("wqkvT", (C, 3 * C), BF)
    woT_d = din("woT", (C, C), BF)
    attWT_d = din("attWT", (C, AWC), BF)
    outWT_d = din("outWT", (C, C), BF)
    peW1T_d = din("peW1T", (3, C), BF)
    peW2T_d = din("peW2T", (C, C), BF)
    ffW1T_d = din("ffW1T", (C, FFD), BF)
    ffW2T_d = din("ffW2T", (FFD, C), BF)
    bq_d = din("bq", (128, 2))
    bk_d = din("bk", (128, 2))
    bv_d = din("bv_b", (128, C))
    bo_d = din("bo_b", (128, C))
    attb_d = din("attb_b", (128, AWC))
    outb_d = din("outb_b", (128, C))
    peb1_d = din("peb1_b", (128, C))
    peb2_d = din("peb2_b", (128, C))
    ffb1_d = din("ffb1", (128, 8))
    ffb2_d = din("ffb2_b", (128, C))
    nrm_d = {k: din(k + "_b", (128, C)) for k in
             ("n1g", "n1b", "n2g", "n2b", "n3g", "n3b",
              "peg1", "pebt1", "peg2", "pebt2")}
    identf_d = din("identf", (128, 128))
    onesf_d = din("onesf", (1, 128))

    out_rows = nc.dram_tensor("out_rows", [QL, C], F32,
                              kind="ExternalOutput").ap()

    ctx = contextlib.ExitStack()
    with ctx:
        # ------------- pools -------------
        cons = ctx.enter_context(tc.tile_pool(name="cons", bufs=1))
        work = ctx.enter_context(tc.tile_pool(name="work", bufs=3))
        esc_p = ctx.enter_context(tc.tile_pool(name="esc", bufs=2))
        ht_p = ctx.enter_context(tc.tile_pool(name="ht", bufs=3))
        ps_tp = ctx.enter_context(tc.tile_pool(name="ps_tp", bufs=3,
                                               space="PSUM"))
        ps_mm = ctx.enter_context(tc.tile_pool(name="ps_mm", bufs=3,
                                               space="PSUM"))
        ps_bc = ctx.enter_context(tc.tile_pool(name="ps_bc", bufs=2,
                                               space="PSUM"))
        bias_p = ctx.enter_context(tc.tile_pool(name="biasp", bufs=4))
        wz_p = ctx.enter_context(tc.tile_pool(name="wzp", bufs=2))

        MM = nc.tensor.matmul

        def lb(ap):
            t = bias_p.tile(list(ap.shape), ap.dtype, name="lb", tag="lb")
            nc.sync.dma_start(out=t[:], in_=ap[:])
            return t

        def lw(ap):
            rows, ncols = ap.shape
            k = rows // 128
            t = wz_p.tile([128, k, ncols], ap.dtype, name="lw", tag="lw")
            srcap = bass.AP(tensor=ap.tensor, offset=0,
                            ap=[[ncols, 128], [128 * ncols, k], [1, ncols]])
            nc.sync.dma_start(out=t[:], in_=srcap)
            return t

        def csb(shape, dt, tag):       # persistent tile (unique tag!)
            return cons.tile(list(shape), dt, name=tag, tag=tag)

        def load(ap, tag):
            t = csb(ap.shape, ap.dtype, tag)
            nc.sync.dma_start(out=t[:], in_=ap[:])
            return t

        ident = load(identf_d, "ident")
        onesf = load(onesf_d, "onesf")
        peW1T = load(peW1T_d, "peW1T")
        bq = load(bq_d, "bq")
        bk = load(bk_d, "bk")
        ffb1 = load(ffb1_d, "ffb1")
        xpos_bf = load(xposbf_d, "xposbf")
        xv_bf = load(xvbf_d, "xvbf")
        xloc_bf = load(xlocbf_d, "xlocbf")
        qrows = csb((128, 2, C), F32, "qrows")
        nc.sync.dma_start(out=qrows[:], in_=bass.AP(
            tensor=qrows_d.tensor, offset=0,
            ap=[[C, 128], [128 * C, 2], [1, C]]))
        refrows = csb((128, 2, 3), F32, "refr")
        nc.sync.dma_start(out=refrows[:], in_=bass.AP(
            tensor=refrows_d.tensor, offset=0,
            ap=[[3, 128], [128 * 3, 2], [1, 3]]))
        hostw = load(hostw_d, "hostw")
        camidx = load(camidx_d, "camidx")
        epst = csb((128, 1), F32, "epst")
        nc.vector.memset(epst[:], EPS)

        # ==================================================================
        # G: DRAM-source gathers, one per camera (start immediately;
        # they run on gpsimd while PE does self-attention).
        # g[n][qp, (l*2+r)*2 + qblk, 0:512] = camf[n][idx, :] || camf[n][idx+1, :]
        # ==================================================================
        g_sb = []
        for n in range(NCAM):
            g = csb((128, NJ, 2 * C), BF, f"g{n}")
            # overlapping view: row stride C, row width 2C (x-corner pair);
            # one row fewer so the widest (last-row) read stays in bounds
            src = bass.AP(tensor=camf_d[n].tensor, offset=0,
                          ap=[[C, CAMROWS - 1], [1, 2 * C]])
            nc.gpsimd.dma_gather(
                out_ap=g[:], in_ap=src, idxs_ap=camidx[:, n, :],
                num_idxs=NIDX, num_idxs_reg=NIDX, elem_size=2 * C,
                elem_step=C, single_packet=False)
            g_sb.append(g)

        # ==================================================================
        # S2: QKV projections (bf16)
        # ==================================================================
        wqkvT = lw(wqkvT_d)
        bv_b = lb(bv_d)
        qhT = csb((128, 2, QL), BF, "qhT")   # head h -> [32*(h%4), h//4, :]
        khT = csb((128, 2, QP), BF, "khT")
        for ch in range(2):
            qp_ = ps_mm.tile([128, QL], F32, name='psmm')
            for kc in range(2):
                MM(qp_[:], wqkvT[:, kc, ts_(ch, 128)],
                   xloc_bf[:, kc, :], start=(kc == 0), stop=(kc == 1))
            nc.vector.tensor_scalar(qhT[:, ch, :], qp_[:],
                                    bq[:, ch:ch + 1], None, AL.add)
            for nn_ in range(2):
                kp = ps_mm.tile([128, 512], F32, name='psmm')
                for kc in range(2):
                    MM(kp[:], wqkvT[:, kc, 256 + ch * 128:256 + (ch + 1) * 128],
                       xpos_bf[:, kc, ts_(nn_, 512)],
                       start=(kc == 0), stop=(kc == 1))
                nc.vector.tensor_scalar(khT[:, ch, ts_(nn_, 512)], kp[:],
                                        bk[:, ch:ch + 1], None, AL.add)
        v_sb = []
        for kt in range(8):
            vt = csb((128, NH, DH + 1), BF, f"vsb{kt}")
            nc.vector.memset(vt[:], 1.0)
            v_sb.append(vt)
        for kt in range(8):
            vp = ps_mm.tile([128, C], F32, name='psmm')
            for kc in range(2):
                MM(vp[:], xv_bf[:, kc, ts_(kt, 128)],
                   wqkvT[:, kc, 512:768],
                   start=(kc == 0), stop=(kc == 1))
            vt = v_sb[kt]
            dst = bass.AP(tensor=vt.tensor, offset=vt[:].offset,
                          ap=[vt[:].ap[0], [DH + 1, NH], [1, DH]])
            nc.vector.tensor_add(dst, vp[:], bv_b[:])

        # ==================================================================
        # S3: attention per head -> oT
        # ==================================================================
        oT = csb((128, 2, QL), BF, "oT")
        for h in range(NH):
            r, chh = 32 * (h % 4), h // 4
            esc = esc_p.tile([128, 8, QL], BF, name="esc", tag="esc")
            for kt in range(8):
                sp = ps_mm.tile([128, QL], F32, name='psmm')
                MM(sp[:], khT[r:r + 32, chh, ts_(kt, 128)],
                   qhT[r:r + 32, chh, :], start=True, stop=True,
                   tile_position=(r, 0))
                if kt == 7:
                    nc.vector.memset(esc[:, 7, :], 0.0)
                    nc.scalar.activation(esc[0:4, 7, :], sp[0:4, :], AF.Exp)
                else:
                    nc.scalar.activation(esc[:, kt, :], sp[:], AF.Exp)
            ov = ps_mm.tile([DH + 1, QL], F32, name='psmm')
            for kt in range(8):
                MM(ov[:], v_sb[kt][:, h, :], esc[:, kt, :],
                   start=(kt == 0), stop=(kt == 7))
            ovs = work.tile([DH + 1, QL], F32, name="ovs", tag="ovs", bufs=2)
            nc.scalar.copy(out=ovs[:], in_=ov[:])
            rinv = work.tile([1, QL], F32, name="rinv", tag="rinv", bufs=2)
            nc.vector.reciprocal(rinv[:], ovs[DH:DH + 1, :])
            bc = ps_bc.tile([DH, QL], F32, name='psbc')
            MM(bc[:], onesf[:, 0:DH], rinv[:], start=True, stop=True)
            nc.vector.tensor_mul(oT[r:r + 32, chh, :], ovs[0:DH, :], bc[:])

        # S4: sa rows + residual + LN1 -> x1 rows; x1T
        x1 = csb((128, 2, C), F32, "x1")

        def layernorm(dst, pre, g_d, b_d):
            g = lb(g_d)
            b = lb(b_d)
            st = work.tile([128, 6], F32, name="lnst", tag="lnst")
            nc.vector.bn_stats(out=st[:], in_=pre)
            mv = work.tile([128, 2], F32, name="lnmv", tag="lnmv")
            nc.vector.bn_aggr(out=mv[:], in_=st[:])
            sd = work.tile([128, 1], F32, name="lnsd", tag="lnsd")
            nc.scalar.activation(sd[:], mv[:, 1:2], AF.Sqrt, bias=epst[:])
            ri = work.tile([128, 1], F32, name="lnri", tag="lnri")
            nc.vector.reciprocal(ri[:], sd[:])
            nc.vector.tensor_scalar_sub(dst, pre, mv[:, 0:1])
            nc.vector.tensor_scalar_mul(dst, dst, ri[:])
            nc.vector.tensor_mul(dst, dst, g[:])
            nc.vector.tensor_add(dst, dst, b[:])

        woT = lw(woT_d)
        bo_b = lb(bo_d)
        for m in range(2):
            sap = ps_mm.tile([128, C], F32, name='psmm')
            for kc in range(2):
                MM(sap[:], oT[:, kc, ts_(m, 128)], woT[:, kc, :],
                   start=(kc == 0), stop=(kc == 1))
            t = work.tile([128, C], F32, name="sar", tag="sar", bufs=1)
            nc.vector.tensor_add(t[:], sap[:], bo_b[:])
            nc.vector.tensor_add(t[:], t[:], qrows[:, m, :])
            layernorm(x1[:, m, :], t[:], nrm_d["n1g"], nrm_d["n1b"])

        x1T = csb((128, 2, QL), BF, "x1T")
        for m in range(2):
            for cc in range(2):
                tp = ps_tp.tile([128, 128], F32, name='pstp')
                nc.tensor.transpose(tp[:], x1[:, m, ts_(cc, 128)], ident[:])
                nc.scalar.copy(out=x1T[:, cc, ts_(m, 128)], in_=tp[:])

        # ==================================================================
        # S5: attention weights -> coefficients
        # aw[qp, m, 18] = sigmoid(x1 @ attW.T + attb);
        # coef[qp, m, ((n*3+l)*2+r)*2+c] = hostw * aw[.., n*3+l]
        # ==================================================================
        attWT = lw(attWT_d)
        attb_b = lb(attb_d)
        awr = csb((128, 2, AWC), F32, "awr")
        for m in range(2):
            ap_ = ps_mm.tile([128, AWC], F32, name='psmm')
            for kc in range(2):
                MM(ap_[:], x1T[:, kc, ts_(m, 128)], attWT[:, kc, :],
                   start=(kc == 0), stop=(kc == 1))
            t = work.tile([128, AWC], F32, name="awt", tag="awt")
            nc.vector.tensor_add(t[:], ap_[:], attb_b[:])
            nc.scalar.activation(awr[:, m, :], t[:], AF.Sigmoid)

        coef = csb((128, 2, NCAM * NL * 4), F32, "coef")
        aw_bc = bass.AP(tensor=awr.tensor, offset=awr[:].offset,
                        ap=[awr[:].ap[0], [AWC, 2], [1, AWC], [0, 4]])

        def v4(t):        # [128, 2, 72] tile -> 4D view [128, 2, 18, 4]
            a = t[:]
            return bass.AP(tensor=a.tensor, offset=a.offset,
                           ap=[a.ap[0], [4 * AWC, 2], [4, AWC], [1, 4]])

        nc.vector.tensor_mul(v4(coef), v4(hostw), aw_bc)

        # ==================================================================
        # S6: combine gathered features:
        # acc[qp, m, ch] = sum_{n,l,r,c} coef[qp,m,(n,l,r,c)] *
        #                  g[n][qp, (l*2+r)*2+m, 256c:256c+256]
        # ==================================================================
        acc = csb((128, 2, C), F32, "acc")
        first = True
        for n in range(NCAM):
            for l in range(NL):
                for r in range(2):
                    for m in range(2):
                        j = (l * 2 + r) * 2 + m
                        for c in range(2):
                            col = ((n * NL + l) * 2 + r) * 2 + c
                            src = g_sb[n][:, j, c * C:(c + 1) * C]
                            if first and r == 0 and c == 0 and n == 0 and l == 0:
                                nc.vector.tensor_scalar_mul(
                                    acc[:, m, :], src,
                                    coef[:, m, col:col + 1])
                            else:
                                nc.vector.scalar_tensor_tensor(
                                    out=acc[:, m, :], in0=src,
                                    scalar=coef[:, m, col:col + 1],
                                    in1=acc[:, m, :],
                                    op0=AL.mult, op1=AL.add)
            first = False

        # transpose acc rows -> fT (channel-partition) for out projection
        fT = csb((128, 2, QL), BF, "fT")
        for m in range(2):
            for cc in range(2):
                tp = ps_tp.tile([128, 128], F32, name='pstp')
                nc.tensor.transpose(tp[:], acc[:, m, ts_(cc, 128)], ident[:])
                nc.scalar.copy(out=fT[:, cc, ts_(m, 128)], in_=tp[:])

        # ==================================================================
        # S7: tail (position encoder, out-proj, LN2, FFN, LN3)
        # ==================================================================
        iref = csb((128, 2, 3), F32, "iref")
        for m in range(2):
            rr = refrows[:, m, :]
            a = work.tile([128, 3], F32, name="pea", tag="pea")
            b2 = work.tile([128, 3], F32, name="peb2t", tag="peb2t")
            nc.vector.tensor_scalar(a[:], rr, EPS, 1.0, AL.max, AL.min)
            nc.vector.tensor_scalar(b2[:], rr, -1.0, 1.0, AL.mult, AL.add)
            nc.vector.tensor_scalar(b2[:], b2[:], EPS, 1.0, AL.max, AL.min)
            rb = work.tile([128, 3], F32, name="perb", tag="perb")
            nc.vector.reciprocal(rb[:], b2[:])
            nc.vector.tensor_mul(a[:], a[:], rb[:])
            nc.scalar.activation(iref[:, m, :], a[:], AF.Ln)
        irT = csb((3, QL), BF, "irT")
        for m in range(2):
            tp = ps_tp.tile([128, 128], F32, name='pstp')
            nc.tensor.transpose(tp[0:3, :], iref[:, m, :], ident[:])
            nc.scalar.copy(out=irT[:, ts_(m, 128)], in_=tp[0:3, :])
        peb1_b = lb(peb1_d)
        pe1 = csb((128, 2, C), F32, "pe1")
        for m in range(2):
            pp = ps_mm.tile([128, C], F32, name='psmm')
            MM(pp[:], irT[:, ts_(m, 128)], peW1T[:], start=True, stop=True)
            t = work.tile([128, C], F32, name="pet", tag="pet", bufs=1)
            nc.vector.tensor_add(t[:], pp[:], peb1_b[:])
            layernorm(pe1[:, m, :], t[:], nrm_d["peg1"], nrm_d["pebt1"])
            nc.scalar.activation(pe1[:, m, :], pe1[:, m, :], AF.Relu)
        pe1T = csb((128, 2, QL), BF, "pe1T")
        for m in range(2):
            for cc in range(2):
                tp = ps_tp.tile([128, 128], F32, name='pstp')
                nc.tensor.transpose(tp[:], pe1[:, m, ts_(cc, 128)], ident[:])
                nc.scalar.copy(out=pe1T[:, cc, ts_(m, 128)], in_=tp[:])

        outWT = lw(outWT_d)
        peW2T = lw(peW2T_d)
        outb_b = lb(outb_d)
        peb2_b = lb(peb2_d)
        x2 = csb((128, 2, C), F32, "x2")
        for m in range(2):
            op_ = ps_mm.tile([128, C], F32, name='psmm')
            for kc in range(2):
                MM(op_[:], fT[:, kc, ts_(m, 128)], outWT[:, kc, :],
                   start=(kc == 0), stop=(kc == 1))
            pp = ps_mm.tile([128, C], F32, name='psmm')
            for kc in range(2):
                MM(pp[:], pe1T[:, kc, ts_(m, 128)], peW2T[:, kc, :],
                   start=(kc == 0), stop=(kc == 1))
            pe2 = work.tile([128, C], F32, name="pe2", tag="pe2", bufs=1)
            nc.vector.tensor_add(pe2[:], pp[:], peb2_b[:])
            layernorm(pe2[:], pe2[:], nrm_d["peg2"], nrm_d["pebt2"])
            nc.scalar.activation(pe2[:], pe2[:], AF.Relu)
            cr = work.tile([128, C], F32, name="cr", tag="cr", bufs=1)
            nc.vector.tensor_add(cr[:], op_[:], outb_b[:])
            nc.vector.tensor_add(cr[:], cr[:], pe2[:])
            nc.vector.tensor_add(cr[:], cr[:], x1[:, m, :])
            layernorm(x2[:, m, :], cr[:], nrm_d["n2g"], nrm_d["n2b"])

        x2T = csb((128, 2, QL), BF, "x2T")
        for m in range(2):
            for cc in range(2):
                tp = ps_tp.tile([128, 128], F32, name='pstp')
                nc.tensor.transpose(tp[:], x2[:, m, ts_(cc, 128)], ident[:])
                nc.scalar.copy(out=x2T[:, cc, ts_(m, 128)], in_=tp[:])

        # FFN
        ffW1T = lw(ffW1T_d)
        ffW2T = lw(ffW2T_d)
        ffb2_b = lb(ffb2_d)
        yps = [ps_mm.tile([128, C], F32, name='psmm') for _ in range(2)]
        for ft in range(8):
            hp = ps_mm.tile([128, QL], F32, name='psmm')
            for kc in range(2):
                MM(hp[:], ffW1T[:, kc, ts_(ft, 128)], x2T[:, kc, :],
                   start=(kc == 0), stop=(kc == 1))
            h_ = ht_p.tile([128, QL], BF, name="hT", tag="hT")
            nc.scalar.activation(h_[:], hp[:], AF.Relu,
                                 bias=ffb1[:, ft:ft + 1])
            for m in range(2):
                MM(yps[m][:], h_[:, ts_(m, 128)], ffW2T[:, ft, :],
                   start=(ft == 0), stop=(ft == 7))
        for m in range(2):
            t = work.tile([128, C], F32, name="yt", tag="yt", bufs=1)
            nc.vector.tensor_add(t[:], yps[m][:], ffb2_b[:])
            nc.vector.tensor_add(t[:], t[:], x2[:, m, :])
            o = work.tile([128, C], F32, name="orow", tag="orow", bufs=1)
            layernorm(o[:], t[:], nrm_d["n3g"], nrm_d["n3b"])
            nc.sync.dma_start(out=out_rows[ts_(m, 128), :], in_=o[:])


# ==========================================================================
# Host side
# ==========================================================================

def _host_inputs(inputs):
    import ml_dtypes
    f32 = np.float32
    bf16 = ml_dtypes.bfloat16

    query = np.asarray(inputs["query"], f32)
    query_pos = np.asarray(inputs["query_pos"], f32)
    ref = np.asarray(inputs["reference_points"], f32)
    lidar = np.asarray(inputs["lidar2img"], f32)
    feats = [np.asarray(inputs[f"feat{l}"], f32) for l in range(NL)]

    xpos = query + query_pos
    scale = 1.0 / math.sqrt(DH)
    Wqkv = np.asarray(inputs["Wqkv"], f32).copy()
    bqkv = np.asarray(inputs["bqkv"], f32)
    Wqkv[:C] *= scale

    def bcastp(v, n=C):
        return np.ascontiguousarray(
            np.broadcast_to(np.asarray(v, f32).reshape(-1)[:n], (128, n)))

    def colmaj(v, ncols):
        return np.ascontiguousarray(np.asarray(v, f32).reshape(ncols, 128).T)

    common = dict(
        wqkvT=np.ascontiguousarray(Wqkv.T).astype(bf16),
        woT=np.ascontiguousarray(np.asarray(inputs["Wo"], f32).T).astype(bf16),
        attWT=np.ascontiguousarray(
            np.asarray(inputs["attW"], f32).T).astype(bf16),
        outWT=np.ascontiguousarray(
            np.asarray(inputs["outW"], f32).T).astype(bf16),
        peW1T=np.ascontiguousarray(
            np.asarray(inputs["peW1"], f32).T).astype(bf16),
        peW2T=np.ascontiguousarray(
            np.asarray(inputs["peW2"], f32).T).astype(bf16),
        ffW1T=np.ascontiguousarray(
            np.asarray(inputs["ffW1"], f32).T).astype(bf16),
        ffW2T=np.ascontiguousarray(
            np.asarray(inputs["ffW2"], f32).T).astype(bf16),
        bq=colmaj(bqkv[:C] * scale, 2),
        bk=colmaj(bqkv[C:2 * C], 2),
        bv_b=bcastp(bqkv[2 * C:]),
        bo_b=bcastp(inputs["bo"]),
        attb_b=bcastp(inputs["attb"], NCAM * NL),
        outb_b=bcastp(inputs["outb"]),
        peb1_b=bcastp(inputs["peb1"]),
        peb2_b=bcastp(inputs["peb2"]),
        ffb1=colmaj(inputs["ffb1"], 8),
        ffb2_b=bcastp(inputs["ffb2"]),
        identf=np.eye(128, dtype=f32),
        onesf=np.ones((1, 128), f32),
    )
    for nm in ("n1g", "n1b", "n2g", "n2b", "n3g", "n3b",
               "peg1", "pebt1", "peg2", "pebt2"):
        common[nm + "_b"] = bcastp(inputs[nm])

    def pad_q(a, axis):
        pad = [(0, 0)] * a.ndim
        pad[axis] = (0, QP - a.shape[axis])
        return np.pad(a, pad)

    # ---- channel-last bf16 feature pyramids, per (batch, camera) ----
    # camf[b][n]: rows = [lvl0 (23200), lvl1 (5800), lvl2 (1450), pad]
    camf = [[None] * NCAM for _ in range(B)]
    for b in range(B):
        for n in range(NCAM):
            parts = [feats[l][b, n].reshape(C, HWL[l]).T for l in range(NL)]
            buf = np.zeros((CAMROWS, C), dtype=bf16)
            buf[:sum(HWL)] = np.concatenate(parts, axis=0).astype(bf16)
            camf[b][n] = buf

    # ---- gather indices + static bilinear/mask weights (host) ----
    # Projection identical to the reference (f32).
    pcs = np.array(PCS, f32)
    pco = np.array(PCO, f32)
    ref3d = ref * pcs + pco                              # (B, Q, 3)
    ref4 = np.concatenate([ref3d, np.ones_like(ref3d[..., :1])], -1)
    cam4 = np.einsum('bnij,bqj->bnqi', lidar, ref4)      # (B, NC, Q, 4)
    z = cam4[..., 2]
    mask = z > EPS
    zc = np.maximum(z, EPS)
    xr = cam4[..., 0] / zc
    yr = cam4[..., 1] / zc
    gx = xr / IMG_W * 2.0 - 1.0
    gy = yr / IMG_H * 2.0 - 1.0
    mask = mask & (gx > -1.0) & (gx < 1.0) & (gy > -1.0) & (gy < 1.0)

    # per level pixel coords; (B, NC, Q)
    idx_all = np.zeros((B, NCAM, NL, 2, QP), np.int16)       # (l, r) rows
    wgt_all = np.zeros((B, NCAM, NL, 2, 2, QP), f32)         # (l, r, c)
    for l, (H, W) in enumerate(LVLS):
        x = np.clip((gx + 1.0) * 0.5 * W - 0.5, -4.0, W + 4.0)
        y = np.clip((gy + 1.0) * 0.5 * H - 0.5, -4.0, H + 4.0)
        x0 = np.floor(x)
        y0 = np.floor(y)
        fx = x - x0
        fy = y - y0
        vx0 = (x0 >= 0) & (x0 <= W - 1)
        vx1 = (x0 >= -1) & (x0 <= W - 2)
        vy0 = (y0 >= 0) & (y0 <= H - 1)
        vy1 = (y0 >= -1) & (y0 <= H - 2)
        # x-pair base: left slot of the gathered pair
        xb = np.clip(x0, 0, W - 1).astype(np.int64)
        # weights for the two slots of the pair:
        #  slot0 = feat[xb], slot1 = feat[xb+1]
        # interior (x0>=0): slot0 = (1-fx)*vx0, slot1 = fx*vx1
        # x0 == -1: xb = 0 = the x1 corner -> slot0 = fx*vx1, slot1 = 0
        wx_s0 = np.where(x0 < 0, fx * vx1, (1.0 - fx) * vx0)
        wx_s1 = np.where(x0 < 0, 0.0, fx * vx1)
        wy = [(1.0 - fy) * vy0, fy * vy1]
        yb = [np.clip(y0, 0, H - 1).astype(np.int64),
              np.clip(y0 + 1, 0, H - 1).astype(np.int64)]
        for r in range(2):
            rows = LVL_BASE[l] + yb[r] * W + xb            # (B, NC, Q)
            idx_all[:, :, l, r, :Q] = rows.astype(np.int16)
            wgt_all[:, :, l, r, 0, :Q] = wy[r] * wx_s0 * mask
            wgt_all[:, :, l, r, 1, :Q] = wy[r] * wx_s1 * mask

    in_maps = []
    for core in range(NCORES):
        b, j = core // 4, core % 4
        q0 = j * QL
        m = dict(common)

        def chunk3(a):  # (256, N) -> (128, 2, N)
            return np.ascontiguousarray(
                a.reshape(2, 128, a.shape[1]).transpose(1, 0, 2))

        m["xposbf"] = chunk3(pad_q(xpos[b].T, 1)).astype(bf16)
        m["xvbf"] = chunk3(pad_q(query[b].T, 1)).astype(bf16)
        m["xlocbf"] = chunk3(
            pad_q(xpos[b].T, 1)[:, q0:q0 + QL]).astype(bf16)
        m["qrows_loc"] = np.ascontiguousarray(
            pad_q(query[b], 0)[q0:q0 + QL])
        m["refrows_loc"] = np.ascontiguousarray(
            pad_q(ref[b], 0)[q0:q0 + QL])
        for n in range(NCAM):
            m[f"camf{n}"] = camf[b][n]
        # indices: position ((l*2+r)*2 + qblk)*128 + p  -> [16, NCAM, 96]
        ci = np.zeros((32, NCAM, NIDX // 16), np.int16)
        hw = np.zeros((128, 2, NCAM * NL * 4), f32)
        for n in range(NCAM):
            pos = np.zeros(NIDX, np.int16)
            for l in range(NL):
                for r in range(2):
                    for qb in range(2):
                        sl = slice(q0 + qb * 128, q0 + (qb + 1) * 128)
                        o = ((l * 2 + r) * 2 + qb) * 128
                        pos[o:o + 128] = idx_all[b, n, l, r, sl]
                        for c in range(2):
                            col = ((n * NL + l) * 2 + r) * 2 + c
                            hw[:, qb, col] = wgt_all[b, n, l, r, c, sl]
            ci[:, n, :] = np.tile(pos.reshape(NIDX // 16, 16).T, (2, 1))
        m["camidx"] = ci
        m["hostw"] = hw
        in_maps.append(m)
    return in_maps


def kernel(**inputs):
    if "nc" not in _BUILT:
        _BUILT["nc"] = _build()
    nc = _BUILT["nc"]
    from concourse import bass_utils
    in_maps = _host_inputs(inputs)
    res = bass_utils.run_bass_kernel_spmd(nc, in_maps,
                                          core_ids=list(range(NCORES)))
    out = np.zeros((B, Q, C), np.float32)
    for core in range(NCORES):
        b, j = core // 4, core % 4
        rows = np.asarray(res.results[core]["out_rows"], np.float32)
        lo = j * QL
        hi = min((j + 1) * QL, Q)
        if lo < Q:
            out[b, lo:hi] = rows[:hi - lo]
    return out
